# revision 1
# baseline (speedup 1.0000x reference)
"""Ragged paged attention (16 seqs x 128 q, GQA 8x4, D=128, pages of 64)
as an 8-core SPMD Trainium2 Bass kernel.

Strategy: the 128 (seq, kv_head) work items are independent. The host
sorts them by KV length and deals them across 8 cores x 16 slots so the
per-slot tile count is identical on every core (static balance, no
collectives). All device inputs are packed into ONE [128, W] bf16
tensor, slot-major, so the whole input loads with contiguous HWDGE
DMAs whose arrival order matches processing order:

  per slot j:  qT [d=128, 512] | kT [d=128, T*128] | v [128, T*132]

  qT columns are (g*128+q), softmax scale folded in.
  kT column order per item: 128 "band" cols (kv[L:L+128]) first, then
    the fully visible kv[0:L], then zero padding to T*128.
  v is stored tile-transposed: partition p holds v[t*128+p, 0:132] at
    cols t*132; col 128 is a 1.0 visibility flag (0 on padding) so the
    PV matmul's 129th column emits the softmax denominator for free.

where L = kv_len - 128. Causality: kv position L+b is visible to query
row q iff q >= b, so the mask is one constant triangular multiply on
the (first) band tile's probabilities done off the critical path.
Padding columns produce exp(0)=1 scores but contribute nothing: their
v rows and visibility flags are zero.

Device per item: scores_T[k,gq] = kT_tile^T @ qT (PSUM f32, chunks of
up to 2 k-tiles, triple-buffered) -> bf16 p -> per group g:
out[q, 0:129] += p_g^T @ v_tile (PSUM accumulators packed two per
bank) -> copy/cast [pv | denom] to bf16 -> DMA out; host does the
final divide + reassembly. Dummy matmuls at the head warm the PE HAM
clock-gate (1.2 -> 2.4GHz) during the DMA ramp, and the output DMAs
ride the gpsimd SWDGE ring early / sync HWDGE late so neither the
input stream nor the end barrier waits on them.

The exp is the scalar-engine bottleneck if done there alone (ACT
streams 1 col/cycle @1.2GHz + ~250cy/instr), so chunks are split
between ACT (true exp) and DVE, where a SINGLE tensor_scalar
i16(s*c1 + c2) written through a bf16 bitcast is a Schraudolph
exponential (bit-trick exp2 in the bf16 exponent field, ~2% rms
error on those chunks; measured end-to-end rel err ~9e-3 vs the
2e-2 budget). Engines are assigned per chunk by a greedy simulation
so neither exp engine ever gates the PE. No max-subtraction needed:
q,k ~ N(0,1) keeps scores O(10), within bf16/exp range.
"""

import numpy as np
import ml_dtypes

S = 16          # sequences
QL = 128        # query tokens per sequence
H = 8           # kv heads
G = 4           # query heads per kv head
D = 128         # head size
KMAX = 1024     # max kv positions per sequence
NC = 8          # NeuronCores
NSLOT = S * H // NC  # 16 work items per core

# Schraudolph exp constants for bf16-via-int16:
#   bf16_bits(exp(s)) ~= i16(s * 128/ln2 + (16256 + C))
SCHRAUD_C1 = 128.0 / float(np.log(2.0))
SCHRAUD_C2 = 16256.0 - 5.75

LAST_RESULTS = None  # BassKernelResults of the last run (for test harness)
TRACE = False        # test harness can flip this for a profiled run


def _prep(q, kv_pages, kv_lens, page_indices):
    bf = ml_dtypes.bfloat16
    sm = 1.0 / np.sqrt(D)

    L = kv_lens.astype(np.int64) - QL                     # [S] fully-visible count
    t_item = (L + 127) // 128 + 1                         # tiles incl. band tile

    items = [(s, h) for s in range(S) for h in range(H)]
    items.sort(key=lambda sh: (-int(t_item[sh[0]]), sh))
    slots = [items[NC * j : NC * (j + 1)] for j in range(NSLOT)]
    # slot order = processing order: three small slots prime the
    # pipeline while the DMA stream ramps (single-chunk slots also
    # build up PV backlog so the PE stays dense once the big slots'
    # QK->exp->QK chains start), then largest-to-smallest, ending on
    # the remaining small slot so the drain tail is short
    order = [NSLOT - 1] + list(range(NSLOT - 2)) + [NSLOT - 2]
    slots = [slots[i] for i in order]
    T = [int(t_item[slots[j][0][0]]) for j in range(NSLOT)]  # max in group

    # packed input layout offsets (columns of the [128, W] tensor);
    # the tri mask constant is packed between slot 0 and slot 1 so the
    # early columns load with single contiguous slabs (fewer of the
    # ~620ns serialized DMA-trigger instructions on the startup path)
    qoff, koff, voff, base = [], [], [], []
    w = 0
    trioff = None
    for j in range(NSLOT):
        if j == 1:
            trioff = w
            w += G * QL
        base.append(w)
        qoff.append(w)
        koff.append(w + G * QL)
        voff.append(w + G * QL + T[j] * 128)
        w += G * QL + T[j] * 128 + T[j] * 132
    base.append(w)
    W = w

    kv = kv_pages[page_indices].reshape(S, KMAX, 2 * H, D)

    in_all = np.zeros((NC, 128, W), bf)
    for j in range(NSLOT):
        t = T[j]
        for c in range(NC):
            s, h = slots[j][c]
            l = int(L[s])
            qs = q[s * QL : (s + 1) * QL, h] * sm          # [QL, G, D]
            in_all[c, :, qoff[j] : qoff[j] + G * QL] = (
                qs.transpose(2, 1, 0).reshape(D, G * QL).astype(bf)
            )
            Kd = kv[s, :, h, :]                            # [KMAX, D]
            ko = koff[j]
            in_all[c, :, ko : ko + 128] = Kd[l : l + 128].T.astype(bf)
            in_all[c, :, ko + 128 : ko + 128 + l] = Kd[:l].T.astype(bf)
            Vd = kv[s, :, H + h, :]
            vs = np.zeros((t * 128, 132), np.float32)
            vs[:128, :D] = Vd[l : l + 128]
            vs[:128, D] = 1.0
            vs[128 : 128 + l, :D] = Vd[:l]
            vs[128 : 128 + l, D] = 1.0
            in_all[c, :, voff[j] : voff[j] + t * 132] = (
                vs.reshape(t, 128, 132).transpose(1, 0, 2).reshape(128, t * 132)
            ).astype(bf)

    tri = (np.arange(QL)[None, :] >= np.arange(128)[:, None]).astype(np.float32)
    tri4 = np.tile(tri, (1, G)).astype(bf)                     # [128, 512]
    in_all[:, :, trioff : trioff + G * QL] = tri4[None]
    return slots, T, (qoff, koff, voff, base, W, trioff), in_all


def _chunk_widths(tj):
    # split T[j] k-tiles into exp chunks of <= 2 tiles: 2-bank score
    # chunks allow 3 PSUM score buffers, so QK(c+3) only waits on
    # exp(c) — a deep enough window that neither exp engine ever
    # gates the PE
    return [2] * (tj // 2) + ([1] if tj % 2 else [])


def _plan_chunks(T):
    """Flat chunk schedule + greedy ACT/DVE exp-engine assignment.

    Simulates the chunk pipeline with simple cost models and picks, for
    each chunk's exp, the engine that keeps the PE from ever waiting on
    a PSUM score buffer (exp(c) must finish before QK(c+2) starts).
    """
    by_slot = []  # per-slot [(j, t0, cw, first, last), ...]
    for j in range(NSLOT):
        t0 = 0
        ws = _chunk_widths(T[j])
        sl = []
        for ci, cw in enumerate(ws):
            sl.append((j, t0, cw, ci == 0, t0 + cw == T[j]))
            t0 += cw
        by_slot.append(sl)

    # QK/exp chunks emit slot-sequentially; PVs follow in the same
    # order two chunk-emissions behind (pv_order), so each chunk's exp
    # runs off the PE's critical path while later QK work streams
    chunks = [c for sl in by_slot for c in sl]
    pv_order = list(chunks)

    # cost models (ns)
    def pe_chunk(cw):          # QK MMs + the PV MMs this chunk pays for
        return cw * (216 + 4 * 57)

    def act_exp(cw):
        return (cw * 512 + 250) / 1.2

    def dve_exp(cw):
        return (cw * 512 + 120) / 0.96

    # extra standing DVE work per slot (CAST) and band chunk (tri)
    CAST_NS = (516 + 120) / 0.96
    TRI_NS = (256 + 58) / 0.96

    engines = []
    act_free = 9500.0    # ACT table loaded ~8.9us into exec; stay off it before
    dve_free = 0.0
    act_busy = 0.0
    dve_busy = 16 * TRI_NS + 30 * CAST_NS  # standing DVE work
    t = 7000.0           # approx first-MM time (after init + first slab)
    for c, (j, t0, cw, first, last) in enumerate(chunks):
        t += pe_chunk(cw)

        def project(eng, af, df):
            # exp on eng; tri + CAST always land on DVE (after the exp)
            if eng == "act":
                af = max(af, t) + act_exp(cw)
                done = af
            else:
                df = max(df, t) + dve_exp(cw)
                done = df
            if first:
                df = max(df, done) + TRI_NS
            if last:
                df = max(df, done) + 2 * CAST_NS
            return af, df, done

        a_af, a_df, a_done = project("act", act_free, dve_free)
        d_af, d_df, d_done = project("dve", act_free, dve_free)
        # balance both engines against the PE frontier: maximize the
        # smaller of the two engines' slack behind the PE
        a_cost = max(a_af - t, a_df - t, 0.0)
        d_cost = max(d_af - t, d_df - t, 0.0)
        if a_cost == d_cost:
            # both keep up with the PE: give the chunk to the engine
            # with less cumulative work so transient hiccups never
            # gate the PE
            a_cost = act_busy + act_exp(cw)
            d_cost = dve_busy + dve_exp(cw)
        if d_cost <= a_cost:
            engines.append("dve")
            act_free, dve_free = d_af, d_df
            dve_busy += dve_exp(cw)
        else:
            engines.append("act")
            act_free, dve_free = a_af, a_df
            act_busy += act_exp(cw)
    # the ACT table load finishes ~9.5us in, just before the first
    # exp is needed — alternate the first chunks across both engines
    # (the greedy's conservative table-load guard would serialize them
    # all on the DVE, stalling QK(c+3) on exp(c))
    for c, eng in enumerate(["act", "dve", "act", "dve"]):
        engines[c] = eng
    # pipeline drain: the last chunks' exp cannot hide behind QK work,
    # so split them across both engines to halve the latency
    for c in range(len(chunks) - 5, len(chunks)):
        engines[c] = "both"
    return chunks, engines, pv_order


def _build(T, layout):
    import concourse.bacc as bacc
    import concourse.tile as tile
    from concourse import mybir

    qoff, koff, voff, base, W, trioff = layout
    dt = mybir.dt
    alu = mybir.AluOpType
    nc = bacc.Bacc("TRN2", target_bir_lowering=False, debug=False, num_devices=NC)
    in_d = nc.dram_tensor("inp", [128, W], dt.bfloat16, kind="ExternalInput")
    out_d = nc.dram_tensor(
        "out", [NSLOT, QL, G * 129], dt.bfloat16, kind="ExternalOutput"
    )

    chunks, engines, pv_order = _plan_chunks(T)

    with tile.TileContext(nc) as tc:
        with (
            tc.tile_pool(name="constp", bufs=1) as constp,
            tc.tile_pool(name="pp", bufs=6) as pp,
            tc.tile_pool(name="osp", bufs=6) as osp,
            tc.tile_pool(name="scp", bufs=3, space="PSUM") as scp,
            tc.tile_pool(name="oup", bufs=1, space="PSUM") as oup,
        ):
            in_sb = constp.tile([128, W], dt.bfloat16, tag="ina", name="ina")
            tri_sb = in_sb[:, trioff : trioff + G * QL]
            # HAM warm-up: the PE clock-gate only opens to 2.4GHz after
            # ~3.4us of sustained matmul activity (it opened at 19us on a
            # profiled run, halving the first ~9us of real matmuls). Burn
            # the DMA-ramp idle time on dummy matmuls over a zeroed tile
            # so the clock is warm when the first real QK issues. gpsimd
            # does the memset — it is the first engine out of init. The
            # same zeroed tile feeds a dummy exp so the ~2.7us ACT table
            # load also overlaps the DMA ramp.
            warmw = constp.tile([128, 256], dt.bfloat16, tag="warmw", name="warmw")
            nc.gpsimd.memset(warmw[:], 0.0)
            warmo = constp.tile([128, 1], dt.bfloat16, tag="warmo", name="warmo")
            nc.scalar.activation(
                warmo[:], warmw[:, 0:1], mybir.ActivationFunctionType.Exp
            )
            warmmm = scp.tile([128, 1024], dt.float32, tag="sc", name="warmmm")
            for wi in range(9):
                nc.tensor.matmul(
                    warmmm[:, 0:256], lhsT=warmw[:, :128], rhs=warmw[:],
                    start=True, stop=True, skip_group_check=True,
                )

            # remaining input slabs on the sync HWDGE ring (the gpsimd
            # SWDGE ring moves data ~4x slower per descriptor and starves
            # the PE if bulk input rides it), ordered by first-use time
            slabs = [(0, base[1]), (base[1], base[2])]   # slot0+tri, slot1
            slabs += [(base[2], base[3]), (base[3], base[4])]
            slabs += [(base[4], base[5]), (base[5], base[6])]
            slabs += [(base[6], base[7]), (base[7], base[8]), (base[8], base[9])]
            slabs += [(base[9], base[10]), (base[10], base[12]),
                      (base[12], base[14]), (base[14], base[NSLOT])]
            for c0, c1 in slabs:
                nc.sync.dma_start(in_sb[:, c0:c1], in_d.ap()[:, c0:c1])

            ogbands = {}

            def emit_pv(j, t0, cw, last, p_sb):
                tj = T[j]
                vo = voff[j]
                if t0 == 0:
                    # two PSUM banks hold the four [128,129] PV accumulators
                    ogbands[j] = [
                        oup.tile(
                            [128, 2 * 129], dt.float32, tag=f"ogb{gb}",
                            name=f"ogb{gb}_{j}", bufs=1,
                        )
                        for gb in range(2)
                    ]
                ogband = ogbands[j]
                outp = [ogband[g // 2][:, (g % 2) * 129 : (g % 2) * 129 + 129]
                        for g in range(G)]
                for ti in range(cw):
                    t = t0 + ti
                    for g in range(G):
                        # start=True clears has_written for the WHOLE bank,
                        # so only the first accumulator in each shared bank
                        # may set it; its partner's first write lands on
                        # cleared (overwrite) state.
                        nc.tensor.matmul(
                            outp[g],
                            lhsT=p_sb[:, ti * 512 + g * 128 : ti * 512 + (g + 1) * 128],
                            rhs=in_sb[:, vo + t * 132 : vo + t * 132 + 129],
                            start=(t == 0 and g % 2 == 0),
                            stop=(t == tj - 1),
                            skip_group_check=True,
                        )
                if last:
                    # copy+cast the unnormalized [pv | denom]; host divides
                    o_sb = osp.tile(
                        [128, G * 129], dt.bfloat16, tag="o", name=f"o{j}"
                    )
                    for gb in range(2):
                        # drain slots: ACT has slack there, so split the
                        # two copies across ACT+DVE to unload the DVE
                        if j >= NSLOT - 3 and gb == 1:
                            nc.scalar.copy(
                                o_sb[:, gb * 258 : (gb + 1) * 258], ogband[gb][:]
                            )
                        else:
                            nc.vector.tensor_copy(
                                o_sb[:, gb * 258 : (gb + 1) * 258], ogband[gb][:]
                            )
                    # out on the gpsimd SWDGE ring: never queued behind the
                    # input slabs on the sync ring (HWDGE queues are strict
                    # FIFO, so an early sync-ring output would complete only
                    # after the whole input stream). The last five slots use
                    # sync — its queues are long empty by then, and SWDGE
                    # moves data slowly enough that a tail output on it
                    # gates the end barrier by ~3us.
                    eng = nc.sync if j >= 6 else nc.gpsimd
                    eng.dma_start(out_d.ap()[j], o_sb[:])

            p_tiles = {}   # (j, t0) -> p_sb
            emit_idx = {}  # (j, t0) -> chunk emission index
            next_pv = 0

            def pop_pvs(upto):
                # PVs pop in pv_order, each 2 chunk-emissions after its
                # own QK/exp so the exp has time to complete off the
                # PE's critical path
                nonlocal next_pv
                while next_pv < len(pv_order):
                    pj, pt0, pcw, _, plast = pv_order[next_pv]
                    e = emit_idx.get((pj, pt0))
                    if e is None or upto < e + 2:
                        break
                    emit_pv(pj, pt0, pcw, plast, p_tiles[(pj, pt0)])
                    next_pv += 1

            for c, (j, t0, cw, first, last) in enumerate(chunks):
                ko = koff[j]
                qt = in_sb[:, qoff[j] : qoff[j] + G * QL]
                sc = scp.tile(
                    [128, cw * G * QL], dt.float32, tag="sc", name=f"sc{j}_{t0}"
                )
                for ti in range(cw):
                    t = t0 + ti
                    nc.tensor.matmul(
                        sc[:, ti * 512 : (ti + 1) * 512],
                        lhsT=in_sb[:, ko + t * 128 : ko + (t + 1) * 128],
                        rhs=qt,
                        start=True,
                        stop=True,
                    )
                p_sb = pp.tile(
                    [128, cw * G * QL], dt.bfloat16, tag="p", name=f"p{j}_{t0}"
                )
                def emit_exp(cols):
                    # Schraudolph exp: bf16 bits of exp(s) via one
                    # int16-converted affine op on the vector engine
                    nc.vector.tensor_scalar(
                        p_sb[:, cols].bitcast(dt.int16),
                        sc[:, cols],
                        SCHRAUD_C1,
                        SCHRAUD_C2,
                        op0=alu.mult,
                        op1=alu.add,
                    )

                if engines[c] == "dve":
                    emit_exp(slice(None))
                elif engines[c] == "both":
                    # pipeline drain: no QK work left to hide the exp
                    # latency, so halve it across both engines
                    half = cw * G * QL // 2
                    nc.scalar.activation(
                        p_sb[:, :half], sc[:, :half],
                        mybir.ActivationFunctionType.Exp,
                    )
                    emit_exp(slice(half, cw * G * QL))
                else:
                    nc.scalar.activation(
                        p_sb[:], sc[:], mybir.ActivationFunctionType.Exp
                    )
                if first:  # band tile is first in each slot: mask on DVE
                    nc.vector.tensor_mul(
                        p_sb[:, 0:512], p_sb[:, 0:512], tri_sb[:]
                    )
                p_tiles[(j, t0)] = p_sb
                emit_idx[(j, t0)] = c
                pop_pvs(c)
            pop_pvs(10 ** 9)
    nc.compile()
    return nc


def kernel(q, kv_pages, kv_lens, page_indices, cu_q_lens, num_seqs):
    global LAST_RESULTS
    from concourse.bass_utils import run_bass_kernel_spmd

    q = np.asarray(q, np.float32)
    kv_pages = np.asarray(kv_pages, np.float32)
    kv_lens = np.asarray(kv_lens)
    page_indices = np.asarray(page_indices)

    slots, T, layout, in_all = _prep(q, kv_pages, kv_lens, page_indices)
    nc = _build(T, layout)

    in_maps = [{"inp": in_all[c]} for c in range(NC)]
    res = run_bass_kernel_spmd(nc, in_maps, core_ids=list(range(NC)), trace=TRACE)
    LAST_RESULTS = res

    out = np.zeros((S * QL, H, G, D), np.float32)
    for c in range(NC):
        o = np.asarray(res.results[c]["out"], np.float32).reshape(
            NSLOT, QL, G, 129
        )
        ov = o[:, :, :, :D] / o[:, :, :, D : D + 1]
        for j in range(NSLOT):
            s, h = slots[j][c]
            out[s * QL : (s + 1) * QL, h] = ov[j]
    return out

